# revision 16
# baseline (speedup 1.0000x reference)
"""Trainium2 Bass kernel for nn_CrossAttention (B=4, C=256, H=W=64).

Sharding: 8 cores = (batch b, branch br). Each core computes its
branch's full 4096-query attention + combine for one batch.

Single fused pipeline:
  - 8 i-blocks of 512 rows x 16 windows of 2 key-chunks. Per window:
    2 strip score matmuls (K=32, tile_position rows 0/32) -> stp PSUM
    f32 [128,2,512]; one ACT Exp -> est bf16 [128,2,512] (SBUF ring).
  - r[i]: DVE pair-sums est planes (bf16 4x mode), then 8 exact ones-
    matmul folds per block into a persistent PSUM row.
  - Wca is folded into the v-projection on the host (W2 = Wca @ Wv),
    so "attended" lives in combine space: the 32 bf16 matmuls per
    (block, c2) accumulate DIRECTLY inside the combine's PSUM group,
    emitted one block later (est ring holds 2 blocks), then
    cp = sum_w,t W2vT^T est + bce*r + Wcx (x1*r); out = sum_c|cp|/r
    via |.| (DVE abs_max) and a ones-matmul into the outp PSUM row.
  - No attended PSUM banks, no separate phases: PSUM = stp ring 2x2
    + combine/projection ring 3 + r/outp row pair 1 = 8 banks exactly.
  - q/k/vT2 projections are "pieces" interleaved into the window loop
    on the combine banks (block 0 has no combine work -> pieces live
    there); xa (own branch) loads first so scores start immediately,
    xb (other branch, for vT2) is only needed once combines begin.
"""

import numpy as np
import ml_dtypes

import concourse.bass as bass
import concourse.bass_isa as bass_isa
import concourse.bacc as bacc
import concourse.tile as tile
import concourse.mybir as mybir
from concourse.bass_utils import run_bass_kernel_spmd

B, C, HH, WW = 4, 256, 64, 64
N = HH * WW          # 4096
CQK = 32
NCORES = 8
NCH = N // 128       # 32 key chunks
NWIN = 16            # windows per block (2 chunks each)
NBLK = 8             # i-blocks of 512

F32 = mybir.dt.float32
BF16 = mybir.dt.bfloat16
AF = mybir.ActivationFunctionType
ALU = mybir.AluOpType


def build_program(nc, tc):
    dram = {}
    for name, shape, dt in [
        ("xa", [2, 128, N], BF16), ("xb", [2, 128, N], BF16),
        ("xc", [2, 128, N], BF16),
        ("wqt", [2, 128, 128], BF16), ("wkt", [2, 128, 128], BF16),
        ("wvt2", [2, 128, C], BF16), ("wctx", [2, 128, C], BF16),
        ("bq", [128, 1], F32), ("bce", [1, 2, 128], BF16),
    ]:
        dram[name] = nc.dram_tensor(name, shape, dt, kind="ExternalInput").ap()
    out_d = nc.dram_tensor("out", [1, N], F32, kind="ExternalOutput").ap()

    import contextlib
    with contextlib.ExitStack() as ctx:
        persist = ctx.enter_context(tc.tile_pool(name="persist", bufs=1))

        wq_sb = persist.tile([128, 2, 128], BF16, tag="wq")
        wk_sb = persist.tile([128, 2, 128], BF16, tag="wk")
        wv2_sb = persist.tile([128, 2, C], BF16, tag="wv2")
        wcx_sb = persist.tile([128, 2, C], BF16, tag="wcx")
        bq_sb = persist.tile([128, 1], F32, tag="bq")
        bce_sb = persist.tile([1, 2, 128], BF16, tag="bce")
        onesb_sb = persist.tile([128, 1], BF16, tag="onesb")
        warm_sb = persist.tile([128, 1], BF16, tag="warm")
        xa_sb = [persist.tile([128, N], BF16, tag=f"xa{kc}",
                              name=f"xa{kc}") for kc in range(2)]
        xb_sb = [persist.tile([128, N], BF16, tag=f"xb{kc}",
                              name=f"xb{kc}") for kc in range(2)]
        xc_sb = [persist.tile([128, N], BF16, tag=f"xc{kc}",
                              name=f"xc{kc}") for kc in range(2)]
        q4_sb = persist.tile([128, N], BF16, tag="q4")
        k4_sb = persist.tile([128, N], BF16, tag="k4")
        vT2_sb = persist.tile([128, NWIN, 2, C], BF16, tag="vt2")

        # ---- input DMAs, in consumption order -------------------------
        nc.sync.dma_start(out=bq_sb, in_=dram["bq"])
        for kc in range(2):
            nc.sync.dma_start(out=wq_sb[:, kc, :], in_=dram["wqt"][kc])
        for kc in range(2):
            nc.sync.dma_start(out=wk_sb[:, kc, :], in_=dram["wkt"][kc])
        nc.sync.dma_start(out=bce_sb, in_=dram["bce"])
        for jh in range(2):          # own-branch input: q/k projections
            for kc in range(2):
                nc.sync.dma_start(
                    out=xa_sb[kc][:, jh * (N // 2):(jh + 1) * (N // 2)],
                    in_=dram["xa"][kc][:, jh * (N // 2):(jh + 1) * (N // 2)])
        for kc in range(2):
            nc.sync.dma_start(out=wv2_sb[:, kc, :], in_=dram["wvt2"][kc])
        for kc in range(2):
            nc.sync.dma_start(out=wcx_sb[:, kc, :], in_=dram["wctx"][kc])
        for kc in range(2):          # other branch: vT2 projection
            nc.sync.dma_start(out=xb_sb[kc], in_=dram["xb"][kc])
        for kc in range(2):          # x1 for the combine
            nc.sync.dma_start(out=xc_sb[kc], in_=dram["xc"][kc])
        nc.vector.memset(onesb_sb, 1.0)
        nc.scalar.activation(warm_sb, onesb_sb, AF.Exp)  # pull ACT table load

        # ---- pools -----------------------------------------------------
        ps_stp = ctx.enter_context(
            tc.tile_pool(name="ps_stp", bufs=3, space="PSUM"))
        ps_cmb = ctx.enter_context(
            tc.tile_pool(name="ps_cmb", bufs=2, space="PSUM"))
        sb = ctx.enter_context(tc.tile_pool(name="work_sb", bufs=1))

        # ---- projection pieces (run on the cmb banks) ------------------
        def q_piece(ib, act=False):
            qp = ps_cmb.tile([128, 512], F32, tag="cmb", bufs=2, name="qp")
            for kc in range(2):
                nc.tensor.matmul(qp, wq_sb[:, kc, :],
                                 xa_sb[kc][:, bass.ts(ib, 512)],
                                 start=(kc == 0), stop=(kc == 1))
            nc.scalar.activation(q4_sb[:, bass.ts(ib, 512)], qp,
                                 AF.Identity, bias=bq_sb)

        def k_piece(jb, act=False):
            kp = ps_cmb.tile([128, 512], F32, tag="cmb", bufs=2, name="kp")
            for kc in range(2):
                nc.tensor.matmul(kp, wk_sb[:, kc, :],
                                 xa_sb[kc][:, bass.ts(jb, 512)],
                                 start=(kc == 0), stop=(kc == 1))
            dst = k4_sb[:, bass.ts(jb, 512)]
            if act:
                nc.scalar.activation(dst, kp, AF.Copy)
            else:
                nc.vector.tensor_copy(dst, kp)

        def v_piece(p, act=False):
            vp = ps_cmb.tile([128, 512], F32, tag="cmb", bufs=2,
                             name="vp").rearrange("q (s c) -> q s c", s=2)
            for s in range(2):
                jc = 2 * p + s
                for kc in range(2):
                    nc.tensor.matmul(vp[:, s, :],
                                     xb_sb[kc][:, bass.ts(jc, 128)],
                                     wv2_sb[:, kc, :],
                                     start=(kc == 0), stop=(kc == 1))
            dst = vT2_sb[:, p, :, :]
            if act:
                nc.scalar.activation(dst.rearrange("q s c -> q (s c)"),
                                     vp.rearrange("q s c -> q (s c)"),
                                     AF.Copy)
            else:
                nc.vector.tensor_copy(dst, vp)

        # ---- combine pieces for a finished block -----------------------
        bstate = {}

        def cp_piece(n, c2):
            st = bstate[n]
            u = ps_cmb.tile([128, 512], F32, tag="cmb", bufs=2, name="cp")
            csl = bass.ds(c2 * 128, 128)
            first = True
            for w in range(NWIN):
                for t in range(2):
                    nc.tensor.matmul(u, vT2_sb[:, w, t, csl],
                                     st["est"][w][:, t, :],
                                     start=first, stop=False)
                    first = False
            nc.tensor.matmul(u, bce_sb[:, c2, :], st["rlb"],
                             start=False, stop=False)
            for kc in range(2):
                nc.tensor.matmul(u, wcx_sb[:, kc, csl], st["x1r"][:, kc, :],
                                 start=False, stop=(kc == 1))
            ab = sb.tile([128, 512], BF16, tag="absb", bufs=4, name="absb")
            nc.scalar.activation(ab, u, AF.Abs)
            st.setdefault("absb", []).append(ab)

        def outp_piece(n):
            st = bstate[n]
            asum = sb.tile([128, 512], BF16, tag="asum", bufs=2, name="asum")
            nc.vector.tensor_tensor(asum, st["absb"][0], st["absb"][1],
                                    ALU.add)
            outs = sb.tile([128, 512], F32, tag="outs", bufs=2, name="outs")
            nc.gpsimd.partition_all_reduce(outs, asum, 128,
                                           bass_isa.ReduceOp.add)
            st["outs"] = outs

        def osb_piece(n):
            st = bstate.pop(n)
            osb = sb.tile([1, 512], F32, tag="osb", bufs=2, name="osb")
            nc.vector.tensor_tensor(osb, st["outs"][0:1, :], st["rr"],
                                    ALU.mult)
            nc.sync.dma_start(out=out_d[:, bass.ts(n, 512)], in_=osb)

        # ---- static piece schedule ------------------------------------
        from collections import defaultdict
        pieces = defaultdict(list)

        def sched(bi, w, fn, *a, **k):
            pieces[(bi, w)].append((fn, a, k))

        for i, jb in enumerate(range(1, 8)):
            sched(0, i, k_piece, jb)                 # b0 w0..w6
        sched(0, 7, q_piece, 1)
        for p in range(16):
            sched(0, 8 + p // 2, v_piece, p)         # b0 w8..w15, 2/window
        for ib in range(2, 8):
            sched(ib - 2, 12, q_piece, ib)           # q(ib) two blocks early
        for n in range(NBLK - 1):
            sched(n + 1, 6, cp_piece, n, 0)
            sched(n + 1, 10, cp_piece, n, 1)
            sched(n + 1, 13, outp_piece, n)
            sched(n + 1, 14, osb_piece, n)

        # pre-loop minimal projections (ACT copies; before EXPs exist)
        q_piece(0, act=True)
        k_piece(0, act=True)

        # ---- main loop -------------------------------------------------
        for n in range(NBLK):
            isl = bass.ts(n, 512)
            st = bstate.setdefault(n, {"est": {}})
            rtree = []   # (level, tile); adjacent equal levels merge
            for w in range(NWIN):
                stp = ps_stp.tile([128, 2, 512], F32, tag="stp", bufs=2,
                                  name="stp")
                for t in range(2):
                    jc = 2 * w + t
                    nc.tensor.matmul(
                        stp[:, t, :],
                        k4_sb[32 * t:32 * (t + 1), bass.ts(jc, 128)],
                        q4_sb[32 * t:32 * (t + 1), isl],
                        start=True, stop=True, tile_position=(32 * t, 0))
                est = sb.tile([128, 2, 512], BF16, tag="est", bufs=34,
                              name="est")
                nc.scalar.activation(est.rearrange("p a n -> p (a n)"),
                                     stp.rearrange("p a n -> p (a n)"),
                                     AF.Exp)
                st["est"][w] = est
                # r: DVE binary-tree sum of est planes (bf16, fast mode)
                rtmp = sb.tile([128, 512], BF16, tag="rtmp", bufs=3,
                               name="rtmp")
                nc.vector.tensor_tensor(rtmp, est[:, 0, :], est[:, 1, :],
                                        ALU.add)
                rtree.append((0, rtmp))
                while len(rtree) >= 2 and rtree[-1][0] == rtree[-2][0]:
                    lv, b_ = rtree.pop()
                    _, a_ = rtree.pop()
                    m = sb.tile([128, 512], BF16, tag=f"rt{lv + 1}",
                                bufs=2 if lv >= 2 else 3, name=f"rt{lv + 1}")
                    nc.vector.tensor_tensor(m, a_, b_, ALU.add)
                    rtree.append((lv + 1, m))
                for fn, a, k in pieces.pop((n, w), ()):
                    fn(*a, **k)
            # block tail: fold r over partitions on gpsimd (f32 internally,
            # result broadcast to every partition)
            (_, rt16), = rtree
            racc = sb.tile([128, 512], F32, tag="racc", bufs=2, name="racc")
            nc.gpsimd.partition_all_reduce(racc, rt16, 128,
                                           bass_isa.ReduceOp.add)
            rr = sb.tile([1, 512], F32, tag="rr", bufs=2, name="rr")
            nc.vector.reciprocal_approx_fast(rr, racc[0:1, :])
            rlb = sb.tile([1, 512], BF16, tag="rlb", bufs=2, name="rlb")
            nc.vector.tensor_copy(rlb, racc[0:1, :])
            x1r = sb.tile([128, 2, 512], BF16, tag="x1r", bufs=2,
                          name="x1r")
            for kc in range(2):
                nc.vector.tensor_tensor(x1r[:, kc, :], xc_sb[kc][:, isl],
                                        racc, ALU.mult)
            st.update(rr=rr, rlb=rlb, x1r=x1r)

        # tail: combine for the last block
        cp_piece(NBLK - 1, 0)
        cp_piece(NBLK - 1, 1)
        outp_piece(NBLK - 1)
        osb_piece(NBLK - 1)


_NC_CACHE = {}


def _get_nc():
    if "nc" not in _NC_CACHE:
        nc = bacc.Bacc("TRN2", debug=False, enable_asserts=False,
                       target_bir_lowering=False, enable_partition_id=False)
        with tile.TileContext(nc) as tc:
            build_program(nc, tc)
        nc.compile()
        _NC_CACHE["nc"] = nc
    return _NC_CACHE["nc"]


def host_inputs(x1, x2, Wq, bq, Wk, bk, Wv, bv, Wc, bc):
    """Build the 8 per-core input maps (host-side sharding/layout only)."""
    f = np.float32
    bf = ml_dtypes.bfloat16
    x1 = np.asarray(x1, f); x2 = np.asarray(x2, f)
    Wq = np.asarray(Wq, f); bq = np.asarray(bq, f)
    Wk = np.asarray(Wk, f)
    Wv = np.asarray(Wv, f); bv = np.asarray(bv, f)
    Wc = np.asarray(Wc, f); bc = np.asarray(bc, f)

    Wq4 = np.tile(Wq, (4, 1))            # [128, 256]
    Wk4 = np.tile(Wk, (4, 1))
    wqt = np.ascontiguousarray(Wq4.T.reshape(2, 128, 128)).astype(bf)
    wkt = np.ascontiguousarray(Wk4.T.reshape(2, 128, 128)).astype(bf)
    bq4 = np.tile(bq, 4).reshape(128, 1).copy()
    Wcx, Wca = Wc[:, :C], Wc[:, C:]
    W2 = Wca @ Wv                        # fold Wca into v projection
    wvt2 = np.ascontiguousarray(W2.T.reshape(2, 128, C)).astype(bf)
    wctx = np.ascontiguousarray(Wcx.T.reshape(2, 128, C)).astype(bf)
    bce = (bc + Wca @ bv).reshape(1, 2, 128).astype(bf)

    xs = [np.ascontiguousarray(x.reshape(B, 2, 128, N)).astype(bf)
          for x in (x1, x2)]
    in_maps = []
    for core in range(NCORES):
        b, br = divmod(core, 2)
        in_maps.append({
            "xa": xs[br][b], "xb": xs[1 - br][b], "xc": xs[0][b],
            "wqt": wqt, "wkt": wkt, "wvt2": wvt2, "wctx": wctx,
            "bq": bq4, "bce": bce,
        })
    return in_maps


def assemble(results):
    """results: 8 dicts with 'out' [1, N] -> (out1, out2) full."""
    outs = []
    for br in range(2):
        full = np.empty((B, 1, HH, WW), np.float32)
        for b in range(B):
            full[b, 0] = results[2 * b + br]["out"][0].reshape(HH, WW)
        outs.append(full)
    return outs[0], outs[1]


def kernel(x1, x2, Wq, bq, Wk, bk, Wv, bv, Wc, bc):
    in_maps = host_inputs(x1, x2, Wq, bq, Wk, bk, Wv, bv, Wc, bc)
    nc = _get_nc()
    res = run_bass_kernel_spmd(nc, in_maps, core_ids=list(range(NCORES)))
    return assemble(res.results)


# revision 17
# speedup vs baseline: 1.1486x; 1.1486x over previous
"""Trainium2 Bass kernel for nn_CrossAttention (B=4, C=256, H=W=64).

Sharding: 8 cores = (batch b, branch br). Each core computes its
branch's full 4096-query attention + combine for one batch.

Single fused pipeline:
  - 8 i-blocks of 512 rows x 16 windows of 2 key-chunks. Per window:
    2 strip score matmuls (K=32, tile_position rows 0/32) -> stp PSUM
    f32 [128,2,512]; one ACT Exp -> est bf16 [128,2,512] (SBUF ring).
  - r[i]: DVE pair-sums est planes (bf16 4x mode), then 8 exact ones-
    matmul folds per block into a persistent PSUM row.
  - Wca is folded into the v-projection on the host (W2 = Wca @ Wv),
    so "attended" lives in combine space: the 32 bf16 matmuls per
    (block, c2) accumulate DIRECTLY inside the combine's PSUM group,
    emitted one block later (est ring holds 2 blocks), then
    cp = sum_w,t W2vT^T est + bce*r + Wcx (x1*r); out = sum_c|cp|/r
    via |.| (DVE abs_max) and a ones-matmul into the outp PSUM row.
  - No attended PSUM banks, no separate phases: PSUM = stp ring 2x2
    + combine/projection ring 3 + r/outp row pair 1 = 8 banks exactly.
  - q/k/vT2 projections are "pieces" interleaved into the window loop
    on the combine banks (block 0 has no combine work -> pieces live
    there); xa (own branch) loads first so scores start immediately,
    xb (other branch, for vT2) is only needed once combines begin.
"""

import numpy as np
import ml_dtypes

import concourse.bass as bass
import concourse.bass_isa as bass_isa
import concourse.bacc as bacc
import concourse.tile as tile
import concourse.mybir as mybir
from concourse.bass_utils import run_bass_kernel_spmd

B, C, HH, WW = 4, 256, 64, 64
N = HH * WW          # 4096
CQK = 32
NCORES = 8
NCH = N // 128       # 32 key chunks
NWIN = 16            # windows per block (2 chunks each)
NBLK = 8             # i-blocks of 512

F32 = mybir.dt.float32
BF16 = mybir.dt.bfloat16
AF = mybir.ActivationFunctionType
ALU = mybir.AluOpType


def build_program(nc, tc):
    dram = {}
    for name, shape, dt in [
        ("xa", [2, 128, N], BF16), ("xb", [2, 128, N], BF16),
        ("xc", [2, 128, N], BF16),
        ("wqt", [2, 128, 128], BF16), ("wkt", [2, 128, 128], BF16),
        ("wvt2", [2, 128, C], BF16), ("wctx", [2, 128, C], BF16),
        ("bq", [128, 1], F32), ("bce", [1, 2, 128], BF16),
    ]:
        dram[name] = nc.dram_tensor(name, shape, dt, kind="ExternalInput").ap()
    out_d = nc.dram_tensor("out", [1, N], F32, kind="ExternalOutput").ap()

    import contextlib
    with contextlib.ExitStack() as ctx:
        persist = ctx.enter_context(tc.tile_pool(name="persist", bufs=1))

        wq_sb = persist.tile([128, 2, 128], BF16, tag="wq")
        wk_sb = persist.tile([128, 2, 128], BF16, tag="wk")
        wv2_sb = persist.tile([128, 2, C], BF16, tag="wv2")
        wcx_sb = persist.tile([128, 2, C], BF16, tag="wcx")
        bq_sb = persist.tile([128, 1], F32, tag="bq")
        bce_sb = persist.tile([1, 2, 128], BF16, tag="bce")
        onesb_sb = persist.tile([128, 1], BF16, tag="onesb")
        warm_sb = persist.tile([128, 1], BF16, tag="warm")
        xa_sb = [persist.tile([128, N], BF16, tag=f"xa{kc}",
                              name=f"xa{kc}") for kc in range(2)]
        xb_sb = [persist.tile([128, N], BF16, tag=f"xb{kc}",
                              name=f"xb{kc}") for kc in range(2)]
        xc_sb = [persist.tile([128, N], BF16, tag=f"xc{kc}",
                              name=f"xc{kc}") for kc in range(2)]
        q4_sb = persist.tile([128, N], BF16, tag="q4")
        k4_sb = persist.tile([128, N], BF16, tag="k4")
        vT2_sb = persist.tile([128, NWIN, 2, C], BF16, tag="vt2")

        # ---- input DMAs, in consumption order -------------------------
        nc.sync.dma_start(out=bq_sb, in_=dram["bq"])
        for kc in range(2):
            nc.sync.dma_start(out=wq_sb[:, kc, :], in_=dram["wqt"][kc])
        for kc in range(2):
            nc.sync.dma_start(out=wk_sb[:, kc, :], in_=dram["wkt"][kc])
        nc.sync.dma_start(out=bce_sb, in_=dram["bce"])
        for jh in range(2):          # own-branch input: q/k projections
            for kc in range(2):
                nc.sync.dma_start(
                    out=xa_sb[kc][:, jh * (N // 2):(jh + 1) * (N // 2)],
                    in_=dram["xa"][kc][:, jh * (N // 2):(jh + 1) * (N // 2)])
        for kc in range(2):
            nc.sync.dma_start(out=wv2_sb[:, kc, :], in_=dram["wvt2"][kc])
        for kc in range(2):
            nc.sync.dma_start(out=wcx_sb[:, kc, :], in_=dram["wctx"][kc])
        for kc in range(2):          # other branch: vT2 projection
            nc.sync.dma_start(out=xb_sb[kc], in_=dram["xb"][kc])
        for kc in range(2):          # x1 for the combine
            nc.sync.dma_start(out=xc_sb[kc], in_=dram["xc"][kc])
        nc.vector.memset(onesb_sb, 1.0)
        nc.scalar.activation(warm_sb, onesb_sb, AF.Exp)  # pull ACT table load

        # ---- pools -----------------------------------------------------
        ps_stp = ctx.enter_context(
            tc.tile_pool(name="ps_stp", bufs=2, space="PSUM"))
        ps_cmb = ctx.enter_context(
            tc.tile_pool(name="ps_cmb", bufs=3, space="PSUM"))
        ps_rp = ctx.enter_context(
            tc.tile_pool(name="ps_rp", bufs=1, space="PSUM"))
        sb = ctx.enter_context(tc.tile_pool(name="work_sb", bufs=1))

        op_t = ps_rp.tile([1, 512], F32, tag="outp")

        # ---- projection pieces (run on the cmb banks) ------------------
        def q_piece(ib, act=False):
            qp = ps_cmb.tile([128, 512], F32, tag="cmb", bufs=3, name="qp")
            for kc in range(2):
                nc.tensor.matmul(qp, wq_sb[:, kc, :],
                                 xa_sb[kc][:, bass.ts(ib, 512)],
                                 start=(kc == 0), stop=(kc == 1))
            nc.scalar.activation(q4_sb[:, bass.ts(ib, 512)], qp,
                                 AF.Identity, bias=bq_sb)

        def k_piece(jb, act=False):
            kp = ps_cmb.tile([128, 512], F32, tag="cmb", bufs=3, name="kp")
            for kc in range(2):
                nc.tensor.matmul(kp, wk_sb[:, kc, :],
                                 xa_sb[kc][:, bass.ts(jb, 512)],
                                 start=(kc == 0), stop=(kc == 1))
            dst = k4_sb[:, bass.ts(jb, 512)]
            if act:
                nc.scalar.activation(dst, kp, AF.Copy)
            else:
                nc.vector.tensor_copy(dst, kp)

        def v_piece(p, act=False):
            vp = ps_cmb.tile([128, 512], F32, tag="cmb", bufs=3,
                             name="vp").rearrange("q (s c) -> q s c", s=2)
            for s in range(2):
                jc = 2 * p + s
                for kc in range(2):
                    nc.tensor.matmul(vp[:, s, :],
                                     xb_sb[kc][:, bass.ts(jc, 128)],
                                     wv2_sb[:, kc, :],
                                     start=(kc == 0), stop=(kc == 1))
            dst = vT2_sb[:, p, :, :]
            if act:
                nc.scalar.activation(dst.rearrange("q s c -> q (s c)"),
                                     vp.rearrange("q s c -> q (s c)"),
                                     AF.Copy)
            else:
                nc.vector.tensor_copy(dst, vp)

        # ---- combine pieces for a finished block -----------------------
        bstate = {}

        def cp_piece(n, c2):
            st = bstate[n]
            u = ps_cmb.tile([128, 512], F32, tag="cmb", bufs=3, name="cp")
            csl = bass.ds(c2 * 128, 128)
            first = True
            for w in range(NWIN):
                for t in range(2):
                    nc.tensor.matmul(u, vT2_sb[:, w, t, csl],
                                     st["est"][w][:, t, :],
                                     start=first, stop=False)
                    first = False
            nc.tensor.matmul(u, bce_sb[:, c2, :], st["rlb"],
                             start=False, stop=False)
            for kc in range(2):
                nc.tensor.matmul(u, wcx_sb[:, kc, csl], st["x1r"][:, kc, :],
                                 start=False, stop=(kc == 1))
            ab = sb.tile([128, 512], BF16, tag="absb", bufs=4, name="absb")
            nc.scalar.activation(ab, u, AF.Abs)
            st.setdefault("absb", []).append(ab)

        def outp_piece(n):
            st = bstate[n]
            for c2 in range(2):
                nc.tensor.matmul(op_t, onesb_sb, st["absb"][c2],
                                 start=(c2 == 0), stop=(c2 == 1))

        def osb_piece(n):
            st = bstate.pop(n)
            osb = sb.tile([1, 512], F32, tag="osb", bufs=2, name="osb")
            nc.vector.tensor_tensor(osb, op_t, st["rr"], ALU.mult)
            nc.sync.dma_start(out=out_d[:, bass.ts(n, 512)], in_=osb)

        # ---- static piece schedule ------------------------------------
        from collections import defaultdict
        pieces = defaultdict(list)

        def sched(bi, w, fn, *a, **k):
            pieces[(bi, w)].append((fn, a, k))

        for i, jb in enumerate(range(1, 8)):
            sched(0, i, k_piece, jb)                 # b0 w0..w6
        sched(0, 7, q_piece, 1)
        for p in range(16):
            sched(0, 8 + p // 2, v_piece, p)         # b0 w8..w15, 2/window
        for ib in range(2, 8):
            sched(ib - 2, 12, q_piece, ib)           # q(ib) two blocks early
        for n in range(NBLK - 1):
            sched(n + 1, 6, cp_piece, n, 0)
            sched(n + 1, 10, cp_piece, n, 1)
            sched(n + 1, 13, outp_piece, n)
            sched(n + 1, 14, osb_piece, n)

        # pre-loop minimal projections (ACT copies; before EXPs exist)
        q_piece(0, act=True)
        k_piece(0, act=True)

        # ---- main loop -------------------------------------------------
        for n in range(NBLK):
            isl = bass.ts(n, 512)
            st = bstate.setdefault(n, {"est": {}})
            rtree = []   # (level, tile); adjacent equal levels merge
            for w in range(NWIN):
                stp = ps_stp.tile([128, 2, 512], F32, tag="stp", bufs=2,
                                  name="stp")
                for t in range(2):
                    jc = 2 * w + t
                    nc.tensor.matmul(
                        stp[:, t, :],
                        k4_sb[32 * t:32 * (t + 1), bass.ts(jc, 128)],
                        q4_sb[32 * t:32 * (t + 1), isl],
                        start=True, stop=True, tile_position=(32 * t, 0))
                est = sb.tile([128, 2, 512], BF16, tag="est", bufs=34,
                              name="est")
                nc.scalar.activation(est.rearrange("p a n -> p (a n)"),
                                     stp.rearrange("p a n -> p (a n)"),
                                     AF.Exp)
                st["est"][w] = est
                # r: DVE binary-tree sum of est planes (bf16, fast mode)
                rtmp = sb.tile([128, 512], BF16, tag="rtmp", bufs=3,
                               name="rtmp")
                nc.vector.tensor_tensor(rtmp, est[:, 0, :], est[:, 1, :],
                                        ALU.add)
                rtree.append((0, rtmp))
                while len(rtree) >= 2 and rtree[-1][0] == rtree[-2][0]:
                    lv, b_ = rtree.pop()
                    _, a_ = rtree.pop()
                    m = sb.tile([128, 512], BF16, tag=f"rt{lv + 1}",
                                bufs=2 if lv >= 2 else 3, name=f"rt{lv + 1}")
                    nc.vector.tensor_tensor(m, a_, b_, ALU.add)
                    rtree.append((lv + 1, m))
                for fn, a, k in pieces.pop((n, w), ()):
                    fn(*a, **k)
            # block tail: fold r over partitions on gpsimd (f32 internally,
            # result broadcast to every partition)
            (_, rt16), = rtree
            racc = sb.tile([128, 512], F32, tag="racc", bufs=2, name="racc")
            nc.gpsimd.partition_all_reduce(racc, rt16, 128,
                                           bass_isa.ReduceOp.add)
            rr = sb.tile([1, 512], F32, tag="rr", bufs=2, name="rr")
            nc.vector.reciprocal_approx_fast(rr, racc[0:1, :])
            rlb = sb.tile([1, 512], BF16, tag="rlb", bufs=2, name="rlb")
            nc.vector.tensor_copy(rlb, racc[0:1, :])
            x1r = sb.tile([128, 2, 512], BF16, tag="x1r", bufs=2,
                          name="x1r")
            for kc in range(2):
                nc.vector.tensor_tensor(x1r[:, kc, :], xc_sb[kc][:, isl],
                                        racc, ALU.mult)
            st.update(rr=rr, rlb=rlb, x1r=x1r)

        # tail: combine for the last block
        cp_piece(NBLK - 1, 0)
        cp_piece(NBLK - 1, 1)
        outp_piece(NBLK - 1)
        osb_piece(NBLK - 1)


_NC_CACHE = {}


def _get_nc():
    if "nc" not in _NC_CACHE:
        nc = bacc.Bacc("TRN2", debug=False, enable_asserts=False,
                       target_bir_lowering=False, enable_partition_id=False)
        with tile.TileContext(nc) as tc:
            build_program(nc, tc)
        nc.compile()
        _NC_CACHE["nc"] = nc
    return _NC_CACHE["nc"]


def host_inputs(x1, x2, Wq, bq, Wk, bk, Wv, bv, Wc, bc):
    """Build the 8 per-core input maps (host-side sharding/layout only)."""
    f = np.float32
    bf = ml_dtypes.bfloat16
    x1 = np.asarray(x1, f); x2 = np.asarray(x2, f)
    Wq = np.asarray(Wq, f); bq = np.asarray(bq, f)
    Wk = np.asarray(Wk, f)
    Wv = np.asarray(Wv, f); bv = np.asarray(bv, f)
    Wc = np.asarray(Wc, f); bc = np.asarray(bc, f)

    Wq4 = np.tile(Wq, (4, 1))            # [128, 256]
    Wk4 = np.tile(Wk, (4, 1))
    wqt = np.ascontiguousarray(Wq4.T.reshape(2, 128, 128)).astype(bf)
    wkt = np.ascontiguousarray(Wk4.T.reshape(2, 128, 128)).astype(bf)
    bq4 = np.tile(bq, 4).reshape(128, 1).copy()
    Wcx, Wca = Wc[:, :C], Wc[:, C:]
    W2 = Wca @ Wv                        # fold Wca into v projection
    wvt2 = np.ascontiguousarray(W2.T.reshape(2, 128, C)).astype(bf)
    wctx = np.ascontiguousarray(Wcx.T.reshape(2, 128, C)).astype(bf)
    bce = (bc + Wca @ bv).reshape(1, 2, 128).astype(bf)

    xs = [np.ascontiguousarray(x.reshape(B, 2, 128, N)).astype(bf)
          for x in (x1, x2)]
    in_maps = []
    for core in range(NCORES):
        b, br = divmod(core, 2)
        in_maps.append({
            "xa": xs[br][b], "xb": xs[1 - br][b], "xc": xs[0][b],
            "wqt": wqt, "wkt": wkt, "wvt2": wvt2, "wctx": wctx,
            "bq": bq4, "bce": bce,
        })
    return in_maps


def assemble(results):
    """results: 8 dicts with 'out' [1, N] -> (out1, out2) full."""
    outs = []
    for br in range(2):
        full = np.empty((B, 1, HH, WW), np.float32)
        for b in range(B):
            full[b, 0] = results[2 * b + br]["out"][0].reshape(HH, WW)
        outs.append(full)
    return outs[0], outs[1]


def kernel(x1, x2, Wq, bq, Wk, bk, Wv, bv, Wc, bc):
    in_maps = host_inputs(x1, x2, Wq, bq, Wk, bk, Wv, bv, Wc, bc)
    nc = _get_nc()
    res = run_bass_kernel_spmd(nc, in_maps, core_ids=list(range(NCORES)))
    return assemble(res.results)


# revision 19
# speedup vs baseline: 1.1531x; 1.0039x over previous
"""Trainium2 Bass kernel for nn_CrossAttention (B=4, C=256, H=W=64).

Sharding: 8 cores = (batch b, branch br). Each core computes its
branch's full 4096-query attention + combine for one batch.

Single fused pipeline:
  - 8 i-blocks of 512 rows x 16 windows of 2 key-chunks. Per window:
    2 strip score matmuls (K=32, tile_position rows 0/32) -> stp PSUM
    f32 [128,2,512]; one ACT Exp -> est bf16 [128,2,512] (SBUF ring).
  - r[i]: DVE pair-sums est planes (bf16 4x mode), then 8 exact ones-
    matmul folds per block into a persistent PSUM row.
  - Wca is folded into the v-projection on the host (W2 = Wca @ Wv),
    so "attended" lives in combine space: the 32 bf16 matmuls per
    (block, c2) accumulate DIRECTLY inside the combine's PSUM group,
    emitted one block later (est ring holds 2 blocks), then
    cp = sum_w,t W2vT^T est + bce*r + Wcx (x1*r); out = sum_c|cp|/r
    via |.| (DVE abs_max) and a ones-matmul into the outp PSUM row.
  - No attended PSUM banks, no separate phases: PSUM = stp ring 2x2
    + combine/projection ring 3 + r/outp row pair 1 = 8 banks exactly.
  - q/k/vT2 projections are "pieces" interleaved into the window loop
    on the combine banks (block 0 has no combine work -> pieces live
    there); xa (own branch) loads first so scores start immediately,
    xb (other branch, for vT2) is only needed once combines begin.
"""

import numpy as np
import ml_dtypes

import concourse.bass as bass
import concourse.bass_isa as bass_isa
import concourse.bacc as bacc
import concourse.tile as tile
import concourse.mybir as mybir
from concourse.bass_utils import run_bass_kernel_spmd

B, C, HH, WW = 4, 256, 64, 64
N = HH * WW          # 4096
CQK = 32
NCORES = 8
NCH = N // 128       # 32 key chunks
NWIN = 16            # windows per block (2 chunks each)
NBLK = 8             # i-blocks of 512

F32 = mybir.dt.float32
BF16 = mybir.dt.bfloat16
AF = mybir.ActivationFunctionType
ALU = mybir.AluOpType


def build_program(nc, tc):
    dram = {}
    for name, shape, dt in [
        ("xa", [2, 128, N], BF16), ("xb", [2, 128, N], BF16),
        ("xc", [2, 128, N], BF16),
        ("wqt", [2, 128, 128], BF16), ("wkt", [2, 128, 128], BF16),
        ("wvt2", [2, 128, C], BF16), ("wctx", [2, 128, C], BF16),
        ("bq", [128, 1], F32), ("bce", [1, 2, 128], BF16),
    ]:
        dram[name] = nc.dram_tensor(name, shape, dt, kind="ExternalInput").ap()
    out_d = nc.dram_tensor("out", [1, N], F32, kind="ExternalOutput").ap()

    import contextlib
    with contextlib.ExitStack() as ctx:
        persist = ctx.enter_context(tc.tile_pool(name="persist", bufs=1))

        wq_sb = persist.tile([128, 2, 128], BF16, tag="wq")
        wk_sb = persist.tile([128, 2, 128], BF16, tag="wk")
        wv2_sb = persist.tile([128, 2, C], BF16, tag="wv2")
        wcx_sb = persist.tile([128, 2, C], BF16, tag="wcx")
        bq_sb = persist.tile([128, 1], F32, tag="bq")
        bce_sb = persist.tile([1, 2, 128], BF16, tag="bce")
        onesb_sb = persist.tile([128, 1], BF16, tag="onesb")
        warm_sb = persist.tile([128, 1], BF16, tag="warm")
        xa_sb = [persist.tile([128, N], BF16, tag=f"xa{kc}",
                              name=f"xa{kc}") for kc in range(2)]
        xb_sb = [persist.tile([128, N], BF16, tag=f"xb{kc}",
                              name=f"xb{kc}") for kc in range(2)]
        xc_sb = [persist.tile([128, N], BF16, tag=f"xc{kc}",
                              name=f"xc{kc}") for kc in range(2)]
        q4_sb = persist.tile([128, N], BF16, tag="q4")
        k4_sb = persist.tile([128, N], BF16, tag="k4")
        vT2_sb = persist.tile([128, NWIN, 2, C], BF16, tag="vt2")

        # ---- input DMAs, in consumption order -------------------------
        nc.sync.dma_start(out=bq_sb, in_=dram["bq"])
        for kc in range(2):
            nc.sync.dma_start(out=wq_sb[:, kc, :], in_=dram["wqt"][kc])
        for kc in range(2):
            nc.sync.dma_start(out=wk_sb[:, kc, :], in_=dram["wkt"][kc])
        nc.sync.dma_start(out=bce_sb, in_=dram["bce"])
        for jh in range(2):          # own-branch input: q/k projections
            for kc in range(2):
                nc.sync.dma_start(
                    out=xa_sb[kc][:, jh * (N // 2):(jh + 1) * (N // 2)],
                    in_=dram["xa"][kc][:, jh * (N // 2):(jh + 1) * (N // 2)])
        for kc in range(2):
            nc.sync.dma_start(out=wv2_sb[:, kc, :], in_=dram["wvt2"][kc])
        for kc in range(2):
            nc.sync.dma_start(out=wcx_sb[:, kc, :], in_=dram["wctx"][kc])
        for kc in range(2):          # other branch: vT2 projection
            nc.sync.dma_start(out=xb_sb[kc], in_=dram["xb"][kc])
        for kc in range(2):          # x1 for the combine
            nc.sync.dma_start(out=xc_sb[kc], in_=dram["xc"][kc])
        nc.vector.memset(onesb_sb, 1.0)
        nc.scalar.activation(warm_sb, onesb_sb, AF.Exp)  # pull ACT table load

        # ---- pools -----------------------------------------------------
        ps_stp = ctx.enter_context(
            tc.tile_pool(name="ps_stp", bufs=2, space="PSUM"))
        ps_cmb = ctx.enter_context(
            tc.tile_pool(name="ps_cmb", bufs=3, space="PSUM"))
        ps_rp = ctx.enter_context(
            tc.tile_pool(name="ps_rp", bufs=1, space="PSUM"))
        sb = ctx.enter_context(tc.tile_pool(name="work_sb", bufs=1))

        op_t = ps_rp.tile([1, 512], F32, tag="outp")

        # ---- projection pieces (run on the cmb banks) ------------------
        def q_piece(ib, act=False):
            qp = ps_cmb.tile([128, 512], F32, tag="cmb", bufs=3, name="qp")
            for kc in range(2):
                nc.tensor.matmul(qp, wq_sb[:, kc, :],
                                 xa_sb[kc][:, bass.ts(ib, 512)],
                                 start=(kc == 0), stop=(kc == 1))
            nc.scalar.activation(q4_sb[:, bass.ts(ib, 512)], qp,
                                 AF.Identity, bias=bq_sb)

        def k_piece(jb, act=False):
            kp = ps_cmb.tile([128, 512], F32, tag="cmb", bufs=3, name="kp")
            for kc in range(2):
                nc.tensor.matmul(kp, wk_sb[:, kc, :],
                                 xa_sb[kc][:, bass.ts(jb, 512)],
                                 start=(kc == 0), stop=(kc == 1))
            dst = k4_sb[:, bass.ts(jb, 512)]
            if act:
                nc.scalar.activation(dst, kp, AF.Copy)
            else:
                nc.vector.tensor_copy(dst, kp)

        def v_piece(p, act=False):
            vp = ps_cmb.tile([128, 512], F32, tag="cmb", bufs=3,
                             name="vp").rearrange("q (s c) -> q s c", s=2)
            for s in range(2):
                jc = 2 * p + s
                for kc in range(2):
                    nc.tensor.matmul(vp[:, s, :],
                                     xb_sb[kc][:, bass.ts(jc, 128)],
                                     wv2_sb[:, kc, :],
                                     start=(kc == 0), stop=(kc == 1))
            dst = vT2_sb[:, p, :, :]
            if act:
                nc.scalar.activation(dst.rearrange("q s c -> q (s c)"),
                                     vp.rearrange("q s c -> q (s c)"),
                                     AF.Copy)
            else:
                nc.vector.tensor_copy(dst, vp)

        # ---- combine pieces for a finished block -----------------------
        bstate = {}

        def cp_att(n, c2, wlo, whi):
            """Attended sub-burst for combine block n, windows [wlo, whi)."""
            st = bstate[n]
            if wlo == 0:
                st[f"u{c2}"] = ps_cmb.tile([128, 512], F32, tag="cmb",
                                           bufs=3, name="cp")
            u = st[f"u{c2}"]
            csl = bass.ds(c2 * 128, 128)
            for w in range(wlo, whi):
                for t in range(2):
                    nc.tensor.matmul(u, vT2_sb[:, w, t, csl],
                                     st["est"][w][:, t, :],
                                     start=(w == 0 and t == 0), stop=False)

        def cp_tail(n, c2):
            st = bstate[n]
            u = st.pop(f"u{c2}")
            csl = bass.ds(c2 * 128, 128)
            nc.tensor.matmul(u, bce_sb[:, c2, :], st["rlb"],
                             start=False, stop=False)
            for kc in range(2):
                nc.tensor.matmul(u, wcx_sb[:, kc, csl], st["x1r"][:, kc, :],
                                 start=False, stop=(kc == 1))
            ab = sb.tile([128, 512], BF16, tag="absb", bufs=4, name="absb")
            nc.scalar.activation(ab, u, AF.Abs)
            st.setdefault("absb", []).append(ab)

        def cp_piece(n, c2):
            cp_att(n, c2, 0, NWIN)
            cp_tail(n, c2)

        def outp_piece(n):
            st = bstate[n]
            for c2 in range(2):
                nc.tensor.matmul(op_t, onesb_sb, st["absb"][c2],
                                 start=(c2 == 0), stop=(c2 == 1))

        def osb_piece(n):
            st = bstate.pop(n)
            osb = sb.tile([1, 512], F32, tag="osb", bufs=2, name="osb")
            nc.vector.tensor_tensor(osb, op_t, st["rr"], ALU.mult)
            nc.sync.dma_start(out=out_d[:, bass.ts(n, 512)], in_=osb)

        # ---- static piece schedule ------------------------------------
        from collections import defaultdict
        pieces = defaultdict(list)

        def sched(bi, w, fn, *a, **k):
            pieces[(bi, w)].append((fn, a, k))

        for i, jb in enumerate(range(1, 8)):
            sched(0, i, k_piece, jb)                 # b0 w0..w6
        sched(0, 7, q_piece, 1)
        for p in range(16):
            sched(0, 8 + p // 2, v_piece, p)         # b0 w8..w15, 2/window
        for ib in range(2, 8):
            sched(ib - 2, 12, q_piece, ib)           # q(ib) two blocks early
        for n in range(NBLK - 1):
            # spread the attended bursts so the PE always has ready work
            for i in range(4):
                sched(n + 1, 1 + i, cp_att, n, 0, 4 * i, 4 * i + 4)
            sched(n + 1, 6, cp_tail, n, 0)
            for i in range(4):
                sched(n + 1, 7 + i, cp_att, n, 1, 4 * i, 4 * i + 4)
            sched(n + 1, 11, cp_tail, n, 1)
            sched(n + 1, 13, outp_piece, n)
            sched(n + 1, 14, osb_piece, n)

        # pre-loop minimal projections (ACT copies; before EXPs exist)
        q_piece(0, act=True)
        k_piece(0, act=True)

        # ---- main loop -------------------------------------------------
        for n in range(NBLK):
            isl = bass.ts(n, 512)
            st = bstate.setdefault(n, {"est": {}})
            rtree = []   # (level, tile); adjacent equal levels merge
            for w in range(NWIN):
                stp = ps_stp.tile([128, 2, 512], F32, tag="stp", bufs=2,
                                  name="stp")
                for t in range(2):
                    jc = 2 * w + t
                    nc.tensor.matmul(
                        stp[:, t, :],
                        k4_sb[32 * t:32 * (t + 1), bass.ts(jc, 128)],
                        q4_sb[32 * t:32 * (t + 1), isl],
                        start=True, stop=True, tile_position=(32 * t, 0))
                est = sb.tile([128, 2, 512], BF16, tag="est", bufs=34,
                              name="est")
                nc.scalar.activation(est.rearrange("p a n -> p (a n)"),
                                     stp.rearrange("p a n -> p (a n)"),
                                     AF.Exp)
                st["est"][w] = est
                # r: DVE binary-tree sum of est planes (bf16, fast mode)
                rtmp = sb.tile([128, 512], BF16, tag="rtmp", bufs=3,
                               name="rtmp")
                nc.vector.tensor_tensor(rtmp, est[:, 0, :], est[:, 1, :],
                                        ALU.add)
                rtree.append((0, rtmp))
                while len(rtree) >= 2 and rtree[-1][0] == rtree[-2][0]:
                    lv, b_ = rtree.pop()
                    _, a_ = rtree.pop()
                    m = sb.tile([128, 512], BF16, tag=f"rt{lv + 1}",
                                bufs=2 if lv >= 2 else 3, name=f"rt{lv + 1}")
                    nc.vector.tensor_tensor(m, a_, b_, ALU.add)
                    rtree.append((lv + 1, m))
                for fn, a, k in pieces.pop((n, w), ()):
                    fn(*a, **k)
            # block tail: fold r over partitions on gpsimd (f32 internally,
            # result broadcast to every partition)
            (_, rt16), = rtree
            racc = sb.tile([128, 512], F32, tag="racc", bufs=2, name="racc")
            nc.gpsimd.partition_all_reduce(racc, rt16, 128,
                                           bass_isa.ReduceOp.add)
            rr = sb.tile([1, 512], F32, tag="rr", bufs=2, name="rr")
            nc.vector.reciprocal_approx_fast(rr, racc[0:1, :])
            rlb = sb.tile([1, 512], BF16, tag="rlb", bufs=2, name="rlb")
            nc.vector.tensor_copy(rlb, racc[0:1, :])
            x1r = sb.tile([128, 2, 512], BF16, tag="x1r", bufs=2,
                          name="x1r")
            for kc in range(2):
                nc.vector.tensor_tensor(x1r[:, kc, :], xc_sb[kc][:, isl],
                                        racc, ALU.mult)
            st.update(rr=rr, rlb=rlb, x1r=x1r)

        # tail: combine for the last block
        cp_piece(NBLK - 1, 0)
        cp_piece(NBLK - 1, 1)
        outp_piece(NBLK - 1)
        osb_piece(NBLK - 1)


_NC_CACHE = {}


def _get_nc():
    if "nc" not in _NC_CACHE:
        nc = bacc.Bacc("TRN2", debug=False, enable_asserts=False,
                       target_bir_lowering=False, enable_partition_id=False)
        with tile.TileContext(nc) as tc:
            build_program(nc, tc)
        nc.compile()
        _NC_CACHE["nc"] = nc
    return _NC_CACHE["nc"]


def host_inputs(x1, x2, Wq, bq, Wk, bk, Wv, bv, Wc, bc):
    """Build the 8 per-core input maps (host-side sharding/layout only)."""
    f = np.float32
    bf = ml_dtypes.bfloat16
    x1 = np.asarray(x1, f); x2 = np.asarray(x2, f)
    Wq = np.asarray(Wq, f); bq = np.asarray(bq, f)
    Wk = np.asarray(Wk, f)
    Wv = np.asarray(Wv, f); bv = np.asarray(bv, f)
    Wc = np.asarray(Wc, f); bc = np.asarray(bc, f)

    Wq4 = np.tile(Wq, (4, 1))            # [128, 256]
    Wk4 = np.tile(Wk, (4, 1))
    wqt = np.ascontiguousarray(Wq4.T.reshape(2, 128, 128)).astype(bf)
    wkt = np.ascontiguousarray(Wk4.T.reshape(2, 128, 128)).astype(bf)
    bq4 = np.tile(bq, 4).reshape(128, 1).copy()
    Wcx, Wca = Wc[:, :C], Wc[:, C:]
    W2 = Wca @ Wv                        # fold Wca into v projection
    wvt2 = np.ascontiguousarray(W2.T.reshape(2, 128, C)).astype(bf)
    wctx = np.ascontiguousarray(Wcx.T.reshape(2, 128, C)).astype(bf)
    bce = (bc + Wca @ bv).reshape(1, 2, 128).astype(bf)

    xs = [np.ascontiguousarray(x.reshape(B, 2, 128, N)).astype(bf)
          for x in (x1, x2)]
    in_maps = []
    for core in range(NCORES):
        b, br = divmod(core, 2)
        in_maps.append({
            "xa": xs[br][b], "xb": xs[1 - br][b], "xc": xs[0][b],
            "wqt": wqt, "wkt": wkt, "wvt2": wvt2, "wctx": wctx,
            "bq": bq4, "bce": bce,
        })
    return in_maps


def assemble(results):
    """results: 8 dicts with 'out' [1, N] -> (out1, out2) full."""
    outs = []
    for br in range(2):
        full = np.empty((B, 1, HH, WW), np.float32)
        for b in range(B):
            full[b, 0] = results[2 * b + br]["out"][0].reshape(HH, WW)
        outs.append(full)
    return outs[0], outs[1]


def kernel(x1, x2, Wq, bq, Wk, bk, Wv, bv, Wc, bc):
    in_maps = host_inputs(x1, x2, Wq, bq, Wk, bk, Wv, bv, Wc, bc)
    nc = _get_nc()
    res = run_bass_kernel_spmd(nc, in_maps, core_ids=list(range(NCORES)))
    return assemble(res.results)
